# revision 1
# baseline (speedup 1.0000x reference)
"""APPNP GNN distributed Bass kernel for TRN2 (8 NeuronCores).

v2 design:
  - Row (destination-node) 1D sharding: core c owns rows [c*R, (c+1)*R).
  - h replicated per-core in DRAM as gather table [N, D] fp32 (256B rows),
    refreshed each APPNP step by a collective AllGather.
  - Edges grouped by destination block (128 rows) with LO/HI split on
    col < N/2 (int16 gather index range); per-(block,side) slots padded to
    a fixed window count (SPMD: same instruction stream on all cores).
  - Messages fetched with gpsimd.dma_gather (fp32), cast to bf16 on the
    scalar engine, and aggregated on the TensorEngine: per 128-edge window
    a CPU-built S matrix [128 edges, 128 dests] (edge weight at the dest
    column) is the stationary operand; PSUM accumulates the segment sum.
    (dma_scatter_add loses concurrent duplicate-row updates -> unusable.)
  - 3-layer MLP on TensorEngine, fp32, activations feature-major.
"""
from contextlib import ExitStack
from dataclasses import dataclass
import numpy as np
import ml_dtypes

from concourse import bass, bacc, mybir, library_config

FP = mybir.dt.float32
BF = mybir.dt.bfloat16
I16 = mybir.dt.int16
AF = mybir.ActivationFunctionType


@dataclass
class Cfg:
    N: int = 65536
    CORES: int = 8
    IN: int = 512           # padded input dim (real 500)
    HID: int = 256
    D: int = 64
    K: int = 10
    ALPHA: float = 0.1
    BPC: int = 2            # dest blocks per chunk
    WLO: int = 0            # windows per block, LO side (filled by prep)
    WHI: int = 0
    DEBUG: bool = False

    @property
    def R(self):
        return self.N // self.CORES

    @property
    def NB(self):           # dest blocks per core
        return self.R // 128

    @property
    def WPB(self):
        return self.WLO + self.WHI

    @property
    def NWIN(self):         # windows per core
        return self.NB * self.WPB

    @property
    def S_SLOTS(self):      # gather slots per core
        return self.NWIN * 128

    @property
    def NCHUNK(self):
        return self.NB // self.BPC


def wrap16(a):
    m = a.reshape(-1, 16).T
    return np.tile(m, (8, 1)).copy()


def prep_inputs(cfg, x, W1, b1, W2, b2, W3, b3, edge_weight, edge_row, edge_col):
    N, R, D = cfg.N, cfg.R, cfg.D
    HALF = N // 2
    edge_row = np.asarray(edge_row).astype(np.int64)
    edge_col = np.asarray(edge_col).astype(np.int64)
    edge_weight = np.asarray(edge_weight).astype(np.float32)
    x = np.asarray(x)

    # global sort once: by (block=row//128, side=col>=HALF)
    blk = edge_row // 128                       # global block id
    side = (edge_col >= HALF).astype(np.int64)
    order = np.lexsort((edge_col, side, blk))
    er, ec, ew, sd = edge_row[order], edge_col[order], edge_weight[order], side[order]
    gblk = blk[order]

    NBG = N // 128                              # total blocks
    cnt = np.zeros((NBG, 2), np.int64)
    np.add.at(cnt, (gblk, sd), 1)
    cfg.WLO = max(int(np.ceil(cnt[:, 0].max() / 128)), 1)
    cfg.WHI = max(int(np.ceil(cnt[:, 1].max() / 128)), 1)

    NB, BPC, WLO, WHI, WPB = cfg.NB, cfg.BPC, cfg.WLO, cfg.WHI, cfg.WPB
    assert NB % BPC == 0
    CH2 = BPC * WPB * 128

    eye = np.eye(128, dtype=np.float32)
    W1p = np.zeros((cfg.IN, cfg.HID), np.float32)
    W1p[:W1.shape[0]] = W1

    # per-edge slot id within its core:
    #   chunk base + LO: brel*WLO*128 + rank | HI: BPC*WLO*128 + brel*WHI*128 + rank
    b_loc = gblk % NB
    chunk = b_loc // BPC
    brel = b_loc % BPC
    grp = gblk * 2 + sd
    grp_starts = np.searchsorted(grp, np.arange(NBG * 2), side="left")
    rank = np.arange(len(er)) - grp_starts[grp]
    slot = (chunk * CH2
            + np.where(sd == 0,
                       brel * WLO * 128 + rank,
                       BPC * WLO * 128 + brel * WHI * 128 + rank))

    core = gblk // NB
    dest_rel = er % 128
    gval = np.where(sd == 0, ec, ec - HALF).astype(np.int16)

    S_SLOTS = cfg.S_SLOTS
    p_arr = (np.arange(S_SLOTS) % 128).astype(np.int64)
    w_arr = (np.arange(S_SLOTS) // 128).astype(np.int64)
    in_maps = []
    for c in range(cfg.CORES):
        m = core == c
        sl = slot[m].astype(np.int64)
        assert sl.max() < S_SLOTS
        gidx = np.zeros(S_SLOTS, np.int16)
        gidx[sl] = gval[m]
        drel = np.zeros(S_SLOTS, np.int64)
        drel[sl] = dest_rel[m]
        wt = np.zeros(S_SLOTS, np.float32)
        wt[sl] = ew[m]

        S = np.zeros((128, cfg.NWIN, 128), ml_dtypes.bfloat16)
        S[p_arr, w_arr, drel] = wt.astype(ml_dtypes.bfloat16)

        xT = np.zeros((cfg.IN, R), np.float32)
        xs = np.asarray(x[c * R:(c + 1) * R])
        xT[:xs.shape[1], :] = xs.T.astype(np.float32)

        in_maps.append({
            "xT": np.ascontiguousarray(xT),
            "W1": W1p,
            "b1": np.asarray(b1).astype(np.float32).reshape(-1, 128).T.copy(),
            "W2": np.asarray(W2).astype(np.float32),
            "b2": np.asarray(b2).astype(np.float32).reshape(-1, 128).T.copy(),
            "W3": np.asarray(W3).astype(np.float32),
            "b3": np.asarray(b3).reshape(-1, 1).astype(np.float32),
            "eye": eye,
            "gidx": wrap16(gidx),
            "smat": S,
        })
    return cfg, in_maps


def build(cfg: Cfg):
    N, R, D, K = cfg.N, cfg.R, cfg.D, cfg.K
    HALF = N // 2
    IN, HID = cfg.IN, cfg.HID
    KIN, KH, MH = IN // 128, HID // 128, HID // 128
    NT = R // 128
    NB, BPC, WLO, WHI, WPB = cfg.NB, cfg.BPC, cfg.WLO, cfg.WHI, cfg.WPB
    NWC = BPC * WPB
    CH2 = NWC * 128
    NLOW = BPC * WLO
    NHIW = BPC * WHI
    NCH = cfg.NCHUNK
    FPB = NB * D

    nc = bacc.Bacc(target_bir_lowering=False, num_devices=cfg.CORES,
                   num_swdge_queues=4)

    xT = nc.declare_dram_parameter("xT", [IN, R], FP, isOutput=False)
    W1 = nc.declare_dram_parameter("W1", [IN, HID], FP, isOutput=False)
    b1 = nc.declare_dram_parameter("b1", [128, HID // 128], FP, isOutput=False)
    W2 = nc.declare_dram_parameter("W2", [HID, HID], FP, isOutput=False)
    b2 = nc.declare_dram_parameter("b2", [128, HID // 128], FP, isOutput=False)
    W3 = nc.declare_dram_parameter("W3", [HID, D], FP, isOutput=False)
    b3 = nc.declare_dram_parameter("b3", [D, 1], FP, isOutput=False)
    eye = nc.declare_dram_parameter("eye", [128, 128], FP, isOutput=False)
    gidx = nc.declare_dram_parameter("gidx", [128, cfg.S_SLOTS // 16], I16, isOutput=False)
    smat = nc.declare_dram_parameter("smat", [128, cfg.NWIN, 128], BF, isOutput=False)
    out = nc.declare_dram_parameter("out", [R, D], FP, isOutput=True)
    dbg_h0 = nc.declare_dram_parameter("dbg_h0", [R, D], FP, isOutput=True) if cfg.DEBUG else None

    table = nc.dram_tensor("table", [N, D], FP, addr_space="Shared")
    hnew = nc.dram_tensor("hnew", [R, D], FP)

    # ---- semaphore plan (every DMA sem has <=1 DMA in flight) ----
    # smain: sync-engine uploads + x tiles (chained)
    # sd0/sd1: S-tile DMAs per parity (chained via matmul-consumption waits)
    # gmain: gpsimd misc DMAs (hnew row writes, h0s, step writes) chained
    # gL0/gL1/gH0/gH1: gather DMAs per parity+side (chained)
    # v/a/p: compute sems (in-order per engine); c: collectives
    MAXW = 8                 # max windows (1024 descs) per dma_gather call
    import math
    NCALL_L = math.ceil(NLOW / MAXW)
    NCALL_H = math.ceil(NHIW / MAXW)
    GATHER_SEMS = [f"g{side}{par}{j}" for side in "LH" for par in range(4)
                   for j in range(NCALL_L if side == "L" else NCALL_H)]
    SEMNAMES = ["smain", "sd0", "sd1", "sd2", "sd3", "gmain"] + GATHER_SEMS + ["v", "a", "p", "c"]
    DMA_SEMS = {"smain", "sd0", "sd1", "sd2", "sd3", "gmain", *GATHER_SEMS}
    ENG_OF = {sn: 'g' for sn in GATHER_SEMS}
    ENG_OF.update({"smain": 's', "sd0": 's', "sd1": 's', "sd2": 's', "sd3": 's', "gmain": 'g',
                   "v": 'v', "a": 'a', "p": 'p', "c": 'g'})
    sched = []      # (engine, fn, waits{semname: val}, semname)
    cnt = {sn: 0 for sn in SEMNAMES}

    def add(semname, fn, waits=None):
        sched.append((ENG_OF[semname], fn, dict(waits or {}), semname))
        cnt[semname] += 16 if semname in DMA_SEMS else 1
        return cnt[semname]

    es = ExitStack()
    with es:
        SEMH = {sn: es.enter_context(nc.semaphore("sem_" + sn)) for sn in SEMNAMES}

        gidx_sb = es.enter_context(nc.sbuf_tensor("gidx_sb", [128, cfg.S_SLOTS // 16], I16))
        msg = [es.enter_context(nc.sbuf_tensor(f"msg{i}", [128, NWC, D], FP)) for i in range(2)]
        msgb = [es.enter_context(nc.sbuf_tensor(f"msgb{i}", [128, NWC, D], BF)) for i in range(2)]
        ssb = [es.enter_context(nc.sbuf_tensor(f"ssb{i}", [128, NWC * 128], BF)) for i in range(2)]
        h0s = es.enter_context(nc.sbuf_tensor("h0s", [128, FPB], FP))
        hnew_sb = es.enter_context(nc.sbuf_tensor("hnew_sb", [128, FPB], FP))
        w1_sb = es.enter_context(nc.sbuf_tensor("w1_sb", [128, KIN, HID], FP))
        w2_sb = es.enter_context(nc.sbuf_tensor("w2_sb", [128, KH, HID], FP))
        w3_sb = es.enter_context(nc.sbuf_tensor("w3_sb", [128, KH, D], FP))
        b1_sb = es.enter_context(nc.sbuf_tensor("b1_sb", [128, MH], FP))
        b2_sb = es.enter_context(nc.sbuf_tensor("b2_sb", [128, MH], FP))
        b3_sb = es.enter_context(nc.sbuf_tensor("b3_sb", [D, 1], FP))
        eye_sb = es.enter_context(nc.sbuf_tensor("eye_sb", [128, 128], FP))
        xt_sb = es.enter_context(nc.sbuf_tensor("xt_sb", [128, KIN, 128], FP))
        h1t_sb = es.enter_context(nc.sbuf_tensor("h1t_sb", [128, KH, 128], FP))
        h2t_sb = es.enter_context(nc.sbuf_tensor("h2t_sb", [128, KH, 128], FP))
        h3t_sb = es.enter_context(nc.sbuf_tensor("h3t_sb", [D, 128], FP))
        h0row_sb = es.enter_context(nc.sbuf_tensor("h0row_sb", [128, D], FP))
        ps_a = es.enter_context(nc.psum_tensor("ps_a", [128, 128], FP))
        ps_b = es.enter_context(nc.psum_tensor("ps_b", [128, 128], FP))
        ps_t = es.enter_context(nc.psum_tensor("ps_t", [128, 128], FP))
        ps_blk = [es.enter_context(nc.psum_tensor(f"ps_blk{i}", [128, D], FP))
                  for i in range(2 * BPC)]
        block = es.enter_context(nc.Block())

        # ---------------- uploads (chained on smain) ----------------
        prev_s = 0
        for fn in (
            lambda s: s.dma_start(out=w1_sb[:, :, :], in_=bass.AP(W1, 0, [[HID, 128], [128 * HID, KIN], [1, HID]])),
            lambda s: s.dma_start(out=w2_sb[:, :, :], in_=bass.AP(W2, 0, [[HID, 128], [128 * HID, KH], [1, HID]])),
            lambda s: s.dma_start(out=w3_sb[:, :, :], in_=bass.AP(W3, 0, [[D, 128], [128 * D, KH], [1, D]])),
            lambda s: s.dma_start(out=b1_sb[:, :], in_=b1[:, :]),
            lambda s: s.dma_start(out=b2_sb[:, :], in_=b2[:, :]),
            lambda s: s.dma_start(out=b3_sb[:, :], in_=b3[:, :]),
            lambda s: s.dma_start(out=eye_sb[:, :], in_=eye[:, :]),
            lambda s: s.dma_start(out=gidx_sb[:, :], in_=gidx[:, :]),
        ):
            prev_s = add("smain", fn, {"smain": prev_s})
        UP_TOT = prev_s

        # ---------------- MLP (single serial chain) ----------------
        prev = ("smain", UP_TOT)

        def chain(semname, fn, extra=None):
            nonlocal prev
            w = {prev[0]: prev[1]}
            if extra:
                for k2, v2 in extra.items():
                    w[k2] = max(w.get(k2, 0), v2)
            val = add(semname, fn, w)
            prev = (semname, val)

        hnw_prev = 0
        for rt in range(NT):
            chain("smain", lambda s, rt=rt: s.dma_start(
                out=xt_sb[:, :, :],
                in_=bass.AP(xT, rt * 128, [[R, 128], [128 * R, KIN], [1, 128]])))
            for ht in range(MH):
                for kc in range(KIN):
                    chain("p", lambda p, ht=ht, kc=kc: p.matmul(
                        ps_a[:, :],
                        bass.AP(w1_sb, kc * HID + ht * 128, [[KIN * HID, 128], [1, 128]]),
                        xt_sb[:, kc, :],
                        start=(kc == 0), stop=(kc == KIN - 1)))
                chain("a", lambda a, ht=ht: a.activation(
                    h1t_sb[:, ht, :], ps_a[:, :], AF.Relu,
                    bias=b1_sb[:, ht:ht + 1], scale=1.0))
            for ht in range(MH):
                for kc in range(KH):
                    chain("p", lambda p, ht=ht, kc=kc: p.matmul(
                        ps_b[:, :],
                        bass.AP(w2_sb, kc * HID + ht * 128, [[KH * HID, 128], [1, 128]]),
                        h1t_sb[:, kc, :],
                        start=(kc == 0), stop=(kc == KH - 1)))
                chain("a", lambda a, ht=ht: a.activation(
                    h2t_sb[:, ht, :], ps_b[:, :], AF.Relu,
                    bias=b2_sb[:, ht:ht + 1], scale=1.0))
            for kc in range(KH):
                chain("p", lambda p, kc=kc: p.matmul(
                    bass.AP(ps_t, 0, [[128, D], [1, 128]]),
                    bass.AP(w3_sb, kc * D, [[KH * D, 128], [1, D]]),
                    h2t_sb[:, kc, :],
                    start=(kc == 0), stop=(kc == KH - 1)))
            chain("v", lambda v: v.tensor_scalar_add(
                h3t_sb[:, :], bass.AP(ps_t, 0, [[128, D], [1, 128]]), b3_sb[:, :]))
            chain("p", lambda p: p.transpose(
                ps_a[:, 0:D], h3t_sb[:, :], eye_sb[0:D, 0:D]))
            chain("a", lambda a: a.activation(
                h0row_sb[:, :], ps_a[:, 0:D], AF.Copy, scale=1.0))
            chain("gmain", lambda g, rt=rt: g.dma_start(
                out=bass.AP(hnew, rt * 128 * D, [[D, 128], [1, D]]),
                in_=h0row_sb[:, :]), extra={"gmain": hnw_prev})
            hnw_prev = cnt["gmain"]

        if cfg.DEBUG:
            chain("gmain", lambda g: g.dma_start(
                out=dbg_h0[:, :], in_=bass.AP(hnew, 0, [[D, R], [1, D]])))

        # h0s = ALPHA * h0  (block-major)
        chain("gmain", lambda g: g.dma_start(
            out=h0s[:, :], in_=bass.AP(hnew, 0, [[D, 128], [128 * D, NB], [1, D]])))
        chain("a", lambda a: a.activation(h0s[:, :], h0s[:, :], AF.Copy, scale=cfg.ALPHA))
        A_H0S = cnt["a"]
        G_MLP = cnt["gmain"]

        # ---------------- APPNP steps ----------------
        conv_done = {}
        mm_after_chunk = {}
        flush_v_after_block = {}
        mm_after_block = {}
        gat_cum = {}
        sd_cum = [0, 0, 0, 0]
        hwr_val = G_MLP
        gci = 0   # global chunk counter across steps

        for k in range(K):
            ag_waits = {"gmain": hwr_val, "a": A_H0S,
                        "sd0": sd_cum[0], "sd1": sd_cum[1]}
            # table reuse: all gathers of previous step done
            for (sidej, par2), val in gat_cum.items():
                ag_waits[f"g{sidej[0]}{par2}{sidej[1:]}"] = val
            add("c", lambda g: g.collective_compute(
                "AllGather", mybir.AluOpType.bypass,
                replica_groups=[list(range(cfg.CORES))],
                ins=[hnew.ap().opt()], outs=[table.ap().opt()]), ag_waits)
            C_NOW = cnt["c"]
            pending_flush = []

            for ci in range(NCH):
                par = gci % 2
                w_g = {"c": C_NOW}
                if conv_done.get(gci - 2) is not None:
                    w_g["a"] = conv_done[gci - 2]
                w_conv_gather = {}
                qrr = 0
                for side, nw_side, base_w, tb_off in (
                    ("L", NLOW, 0, 0), ("H", NHIW, NLOW, HALF * D),
                ):
                    ncall = math.ceil(nw_side / MAXW)
                    for j in range(ncall):
                        w0 = j * MAXW
                        w1 = min(w0 + MAXW, nw_side)
                        sn = f"g{side}{par}{j}"
                        qn = qrr % 4
                        qrr += 1
                        gv = add(sn, lambda g, par=par, w0=w0, w1=w1, base_w=base_w,
                                 tb_off=tb_off, ci=ci, qn=qn: g.dma_gather(
                            out_ap=msg[par][:, base_w + w0:base_w + w1, :],
                            in_ap=bass.AP(table, tb_off, [[D, HALF], [1, D]]),
                            idxs_ap=gidx_sb[:, (ci * CH2 + (base_w + w0) * 128) // 16:
                                            (ci * CH2 + (base_w + w1) * 128) // 16],
                            num_idxs=(w1 - w0) * 128, num_idxs_reg=(w1 - w0) * 128,
                            elem_size=D, queue_num=qn,
                            single_packet=False), w_g)
                        gat_cum[(side + str(j), par)] = gv
                        w_conv_gather[sn] = gv
                w_s = {}
                if mm_after_chunk.get(gci - 2) is not None:
                    w_s["p"] = mm_after_chunk[gci - 2]
                sdv = add("sd" + str(par), lambda s, ci=ci, par=par: s.dma_start(
                    out=ssb[par][:, :],
                    in_=smat[:, ci * NWC:(ci + 1) * NWC, :]), w_s)
                sd_cum[par] = sdv
                w_c = dict(w_conv_gather)
                if mm_after_chunk.get(gci - 2) is not None:
                    w_c["p"] = mm_after_chunk[gci - 2]
                cv = add("a", lambda a, par=par: a.activation(
                    msgb[par][:, :, :], msg[par][:, :, :], AF.Copy, scale=1.0), w_c)
                conv_done[gci] = cv

                for brel in range(BPC):
                    b = ci * BPC + brel
                    gb = k * NB + b
                    psum = ps_blk[((gb // BPC) % 2) * BPC + brel]
                    wins = ([brel * WLO + j for j in range(WLO)]
                            + [NLOW + brel * WHI + j for j in range(WHI)])
                    for wi, w in enumerate(wins):
                        waits = {}
                        if wi == 0:
                            waits = {"a": cv, "sd" + str(par): sdv}
                            prev_gb = gb - 2 * BPC
                            if prev_gb in flush_v_after_block:
                                waits["v"] = flush_v_after_block[prev_gb]
                        add("p", lambda p, par=par, w=w, psum=psum, wi=wi, nw=len(wins): p.matmul(
                            psum[:, :],
                            bass.AP(ssb[par], w * 128, [[NWC * 128, 128], [1, 128]]),
                            bass.AP(msgb[par], w * D, [[NWC * D, 128], [1, D]]),
                            start=(wi == 0), stop=(wi == nw - 1)), waits)
                    mm_after_block[gb] = cnt["p"]
                    pending_flush.append((gb, b, psum))
                    if len(pending_flush) > 1:
                        fgb, fb, fpsum = pending_flush.pop(0)
                        fv = add("v", lambda v, fb=fb, fpsum=fpsum: v.scalar_tensor_tensor(
                            hnew_sb[:, fb * D:(fb + 1) * D], fpsum[:, :],
                            1.0 - cfg.ALPHA, h0s[:, fb * D:(fb + 1) * D],
                            mybir.AluOpType.mult, mybir.AluOpType.add),
                            {"p": mm_after_block[fgb], "a": A_H0S})
                        flush_v_after_block[fgb] = fv
                mm_after_chunk[gci] = cnt["p"]
                gci += 1

            while pending_flush:
                fgb, fb, fpsum = pending_flush.pop(0)
                fv = add("v", lambda v, fb=fb, fpsum=fpsum: v.scalar_tensor_tensor(
                    hnew_sb[:, fb * D:(fb + 1) * D], fpsum[:, :],
                    1.0 - cfg.ALPHA, h0s[:, fb * D:(fb + 1) * D],
                    mybir.AluOpType.mult, mybir.AluOpType.add),
                    {"p": mm_after_block[fgb], "a": A_H0S})
                flush_v_after_block[fgb] = fv

            dst = out if k == K - 1 else hnew
            hwr_val = add("gmain", lambda g, dst=dst: g.dma_start(
                out=bass.AP(dst, 0, [[D, 128], [128 * D, NB], [1, D]]),
                in_=hnew_sb[:, :]), {"v": cnt["v"], "gmain": hwr_val})

        # ------------- emit -------------
        def walk(name):
            def run(eng):
                if name == 'g':
                    eng.load_library(library_config.mlp)
                last = {sn: 0 for sn in SEMNAMES}
                for (e, fn, waits, semname) in sched:
                    if e != name:
                        continue
                    for sk, val in waits.items():
                        if val > last[sk]:
                            eng.wait_ge(SEMH[sk], int(val))
                            last[sk] = int(val)
                    inc = 16 if semname in DMA_SEMS else 1
                    fn(eng).then_inc(SEMH[semname], inc)
                if name == 'g':
                    for sn in SEMNAMES:
                        if cnt[sn] > last[sn]:
                            eng.wait_ge(SEMH[sn], int(cnt[sn]))
            return run

        block.gpsimd(walk('g'))
        block.vector(walk('v'))
        block.sync(walk('s'))
        block.tensor(walk('p'))
        block.scalar(walk('a'))

    return nc


def reference_np(cfg, x, W1, b1, W2, b2, W3, b3, edge_weight, edge_row, edge_col):
    h = np.maximum(x @ W1 + b1, 0)
    h = np.maximum(h @ W2 + b2, 0)
    h = h @ W3 + b3
    h0 = h
    for _ in range(cfg.K):
        msg = h[edge_col] * edge_weight[:, None]
        aggv = np.zeros_like(h0)
        np.add.at(aggv, edge_row, msg)
        h = (1.0 - cfg.ALPHA) * aggv + cfg.ALPHA * h0
    return h


# ----------------------------------------------------------------------------
# Harness entry point: full inputs in, full output out.
# ----------------------------------------------------------------------------
def kernel(**inputs):
    cfg = Cfg()  # full-size defaults
    cfg, in_maps = prep_inputs(
        cfg,
        inputs["x"], inputs["W1"], inputs["b1"], inputs["W2"], inputs["b2"],
        inputs["W3"], inputs["b3"], inputs["edge_weight"],
        inputs["edge_row"], inputs["edge_col"],
    )
    nc = build(cfg)
    nc.finalize()
    from concourse.bass_utils import run_bass_kernel_spmd
    res = run_bass_kernel_spmd(nc, in_maps, core_ids=list(range(cfg.CORES)))
    outs = res.results
    return np.concatenate([o["out"] for o in outs], axis=0).astype(np.float32)

